# revision 7
# baseline (speedup 1.0000x reference)
"""Self-contained Trainium2 Bass kernel for a 2-layer GCN encoder
(PyG GCNConv x2 with LeakyReLU), distributed over 8 NeuronCores.

kernel(**inputs) takes the full unsharded inputs (X [50000,512] f32,
edge_index [2,800000] int64, W1/b1/W2/b2) and returns the full
[50000,128] f32 output. See build() for the device program.

v2 changes vs baseline:
- host-side load balancing: nodes are assigned to 128-row dst blocks so
  per-(block, src-half) edge counts are even -> fewer gather chunks
- one dma_gather call per (group, side) (amortizes the ~1us fixed SWDGE
  cost; needs a larger dynamic-dma descriptor ring)
- z sinks use a single fused Lrelu activation on the Scalar engine
- dense2+allgather for each z1 half are issued inside the layer-1
  message-passing loop so the collectives overlap compute
- A-side gathers run a few groups ahead of B-side gathers so a pending
  allgather of the B table never stalls the gpsimd gather stream
"""

import sys
if "/opt/trn_rl_repo" not in sys.path:
    sys.path.insert(0, "/opt/trn_rl_repo")

import math
from dataclasses import dataclass, field

import numpy as np
import ml_dtypes

import concourse.bass as bass
import concourse.tile as tile
from concourse import bacc, mybir
from concourse.bass_utils import run_bass_kernel_spmd

FP32 = mybir.dt.float32
BF16 = mybir.dt.bfloat16
I32 = mybir.dt.int32
I16 = mybir.dt.int16


@dataclass
class Cfg:
    n: int          # real node count
    e: int          # real edge count
    d_in: int
    h1: int
    h2: int
    cores: int = 8
    bpc: int = 50   # 128-node dst blocks per core
    grp: int = 2    # dst blocks per dma_gather call
    neg: float = 0.2
    look: int = 3   # A-gather lookahead (groups)
    shared_ag: bool = False
    scratch: int = 16384  # dynamic dma descriptor ring: scratch//16 descs
    mcall: int = 8        # max 128-idx chunks per dma_gather call

    @property
    def npad(self):
        return self.cores * self.bpc * 128

    @property
    def shard(self):
        return self.bpc * 128


@dataclass
class Meta:
    cpa: int  # chunks per block, side A (src half 0)
    cpb: int  # chunks per block, side B
    # per-group idx-tile column offsets: list of (g0, gn, colA, colB)
    groups: list = field(default_factory=list)
    tot_cols: int = 0   # idx tile columns (int16 packed by 16)
    nch: int = 0        # chunks per block total


def _balance_slots(cfg: Cfg, src, dst):
    """Assign nodes to slots so per-(block, src-half) in-edge counts are
    balanced. Returns slot_of_node [n] -> slot in [0, npad)."""
    n, npad = cfg.n, cfg.npad
    nblk = npad // 128
    hb = cfg.bpc // 2
    half_blocks = nblk // 2  # blocks 0..half_blocks-1 are half A

    indeg = np.bincount(dst, minlength=n).astype(np.int64)

    # step 1: split nodes into two halves with ~equal total indegree
    order = np.argsort(-indeg, kind="stable")
    half_of_node = np.zeros(n, np.int8)
    # snake: 0,1,1,0,0,1,1,0...
    snake = np.tile([0, 1, 1, 0], n // 4 + 1)[:n]
    half_of_node[order] = snake
    # fix cardinality: each half can hold npad//2 slots; count real nodes
    cnt0 = int((half_of_node == 0).sum())
    cap = npad // 2
    if cnt0 > cap:
        movers = np.where(half_of_node == 0)[0][: cnt0 - cap]
        half_of_node[movers] = 1
    elif n - cnt0 > cap:
        movers = np.where(half_of_node == 1)[0][: (n - cnt0) - cap]
        half_of_node[movers] = 0

    # per-dst-node in-edge counts split by src half (+1 self edge, own half)
    sh = half_of_node[src]
    a_n = np.bincount(dst[sh == 0], minlength=n).astype(np.int64)
    b_n = np.bincount(dst[sh == 1], minlength=n).astype(np.int64)

    slot_of_node = np.empty(n, np.int64)
    # physical block ids: half A blocks 0..199 (core nb%8, p=nb//8 < hb)
    for H in (0, 1):
        nodes = np.where(half_of_node == H)[0]
        aa = a_n[nodes] + (1 if H == 0 else 0)   # self edge on own side
        bb = b_n[nodes] + (1 if H == 1 else 0)
        w = aa + bb
        srt = np.argsort(-w, kind="stable")
        nodes, aa, bb = nodes[srt], aa[srt], bb[srt]
        nb0 = 0 if H == 0 else half_blocks
        cA = np.zeros(half_blocks, np.int64)
        cB = np.zeros(half_blocks, np.int64)
        fill = np.zeros(half_blocks, np.int64)
        lane = np.empty(len(nodes), np.int64)
        blk = np.empty(len(nodes), np.int64)
        for i in range(len(nodes)):
            score = np.maximum(cA + aa[i], cB + bb[i]).astype(np.float64)
            score[fill >= 128] = np.inf
            j = int(np.argmin(score))
            blk[i] = j
            lane[i] = fill[j]
            fill[j] += 1
            cA[j] += aa[i]
            cB[j] += bb[i]
        slot_of_node[nodes] = (nb0 + blk) * 128 + lane
    return slot_of_node


def preprocess(cfg: Cfg, X, edge_index, W1, b1, W2, b2):
    """Host-side: shard + edge partitioning. Returns (in_maps, meta)."""
    n, npad = cfg.n, cfg.npad
    src = np.asarray(edge_index[0], dtype=np.int64)
    dst = np.asarray(edge_index[1], dtype=np.int64)
    assert not np.any(np.asarray(b1)) and not np.any(np.asarray(b2)), \
        "nonzero bias unsupported in this build"

    slot = _balance_slots(cfg, src, dst)

    deg_n = np.bincount(dst, minlength=n).astype(np.float32) + 1.0
    dinv = np.ones(npad, np.float32)
    dinv[slot] = (1.0 / np.sqrt(deg_n)).astype(np.float32)

    # self loops for every real node; edges in slot space
    asrc = slot[np.concatenate([src, np.arange(n, dtype=np.int64)])]
    adst = slot[np.concatenate([dst, np.arange(n, dtype=np.int64)])]

    # gather-table row of a slot: tables are the AG concat of core shards
    hb = cfg.bpc // 2
    nb_ = np.arange(npad, dtype=np.int64) >> 7
    c_ = nb_ % cfg.cores
    p_ = nb_ // cfg.cores
    half_ = (p_ >= hb).astype(np.int64)
    perm = (half_ * (npad // 2) + c_ * (hb * 128) + (p_ - half_ * hb) * 128
            + (np.arange(npad, dtype=np.int64) & 127))
    asrc = perm[asrc]

    blk = adst >> 7
    side = (asrc >= npad // 2).astype(np.int64)
    order = np.lexsort((asrc, side, blk))
    asrc, adst, blk, side = asrc[order], adst[order], blk[order], side[order]

    nblk = npad // 128
    cnt_a = np.bincount(blk[side == 0], minlength=nblk)
    cnt_b = np.bincount(blk[side == 1], minlength=nblk)
    cpa = int(math.ceil(cnt_a.max() / 128)) if cnt_a.max() > 0 else 0
    cpb = int(math.ceil(cnt_b.max() / 128)) if cnt_b.max() > 0 else 0
    cap_a, cap_b = cpa * 128, cpb * 128
    nch = cpa + cpb

    nrows_h = npad // 2
    spread = (np.arange(max(cap_a, cap_b, 1), dtype=np.int64) * 67)
    idx_a = np.ascontiguousarray(
        ((np.arange(nblk)[:, None] * 997 + spread[None, :cap_a]) % nrows_h
         ).astype(np.int16)) if cap_a else np.zeros((nblk, 1), np.int16)
    idx_b = np.ascontiguousarray(
        ((np.arange(nblk)[:, None] * 997 + spread[None, :cap_b]) % nrows_h
         ).astype(np.int16)) if cap_b else np.zeros((nblk, 1), np.int16)
    assert cfg.bpc % 2 == 0
    dstloc = np.full((nblk, nch * 128), -1, np.int32)

    for s, (idx_t, cnt, coff, roff) in enumerate(
            ((idx_a, cnt_a, 0, 0), (idx_b, cnt_b, cap_a, npad // 2))):
        mask = side == s
        b_, s_, d_ = blk[mask], asrc[mask], adst[mask]
        start = np.zeros(nblk + 1, np.int64)
        np.cumsum(cnt, out=start[1:])
        pos = np.arange(len(b_)) - start[b_]
        idx_t[b_, pos] = (s_ - roff).astype(np.int16)
        dstloc[b_, coff + pos] = (d_ & 127).astype(np.int32)

    # group layout for gather calls (identical structure on every core)
    groups = []
    col = 0
    for g0 in range(0, cfg.bpc, cfg.grp):
        gn = min(cfg.grp, cfg.bpc - g0)
        col_a = col
        col_b = col + gn * cap_a // 16
        col = col_b + gn * cap_b // 16
        groups.append((g0, gn, col_a, col_b))
    tot_cols = col

    meta = Meta(cpa=cpa, cpb=cpb, groups=groups, tot_cols=tot_cols, nch=nch)

    # replicated / per-core tensors
    XT = np.zeros((cfg.d_in, npad), np.float32)
    XT[:, slot] = np.asarray(X, np.float32).T
    XT = XT.astype(ml_dtypes.bfloat16)
    W1b = np.asarray(W1, np.float32).astype(ml_dtypes.bfloat16)
    W2b = np.asarray(W2, np.float32).astype(ml_dtypes.bfloat16)
    mxc = max(cpa, cpb, 1)
    iota4 = np.ascontiguousarray(np.broadcast_to(
        np.arange(128, dtype=np.float32)[None, None, :],
        (128, mxc, 128))).astype(ml_dtypes.bfloat16)

    in_maps = []
    for c in range(cfg.cores):
        blocks = list(range(c, nblk, cfg.cores))   # round-robin assignment
        parts = []
        for (g0, gn, _ca, _cb) in groups:
            bsel = blocks[g0:g0 + gn]
            parts.append(idx_a[bsel, :cap_a].reshape(-1))
            parts.append(idx_b[bsel, :cap_b].reshape(-1))
        flat = np.concatenate(parts) if parts else np.zeros(0, np.int16)
        assert flat.size == tot_cols * 16, (flat.size, tot_cols * 16)
        idx_tile = np.ascontiguousarray(
            np.tile(flat.reshape(-1, 16).T, (8, 1)))          # [128, tot_cols]

        dst_tile = np.ascontiguousarray(
            dstloc[blocks].reshape(cfg.bpc * nch, 128).T)      # [128, bpc*nch]

        node_sel = (np.asarray(blocks)[:, None] * 128
                    + np.arange(128)[None, :]).reshape(-1)
        dv = dinv[node_sel].reshape(cfg.bpc, 128).T
        m = {
            "xt": np.ascontiguousarray(XT[:, node_sel]),
            "w1": W1b, "w2": W2b,
            "idx": idx_tile,
            "dstloc": dst_tile.astype(np.float32).astype(ml_dtypes.bfloat16),
            "dinv": np.ascontiguousarray(dv).astype(np.float32),
            "dinv08": np.ascontiguousarray(dv * (1.0 - cfg.neg)).astype(np.float32),
            "dinv02": np.ascontiguousarray(dv * cfg.neg).astype(np.float32),
            "iota4": iota4,
        }
        in_maps.append(m)
    return in_maps, meta, slot


def build(cfg: Cfg, meta: Meta, stop_after: str = 'full'):
    nc = bacc.Bacc("TRN2", target_bir_lowering=False, debug=False,
                   num_devices=cfg.cores, num_swdge_queues=4,
                   dynamic_dma_scratch_size=cfg.scratch)
    sh, npad = cfg.shard, cfg.npad
    kin, kh1 = cfg.d_in // 128, cfg.h1 // 128
    cpa, cpb, nch = meta.cpa, meta.cpb, meta.nch
    mxc = max(cpa, cpb, 1)
    nrows_h = npad // 2
    hb = cfg.bpc // 2
    AT = mybir.ActivationFunctionType
    OP = mybir.AluOpType
    aspace = "Shared" if cfg.shared_ag else "Local"

    xt = nc.dram_tensor("xt", [cfg.d_in, sh], BF16, kind="ExternalInput")
    w1 = nc.dram_tensor("w1", [cfg.d_in, cfg.h1], BF16, kind="ExternalInput")
    w2 = nc.dram_tensor("w2", [cfg.h1, cfg.h2], BF16, kind="ExternalInput")
    idx = nc.dram_tensor("idx", [128, meta.tot_cols], I16, kind="ExternalInput")
    dstloc = nc.dram_tensor("dstloc", [128, cfg.bpc * nch], BF16, kind="ExternalInput")
    dinv = nc.dram_tensor("dinv", [128, cfg.bpc], FP32, kind="ExternalInput")
    dinv08 = nc.dram_tensor("dinv08", [128, cfg.bpc], FP32, kind="ExternalInput")
    dinv02 = nc.dram_tensor("dinv02", [128, cfg.bpc], FP32, kind="ExternalInput")
    iota_d = nc.dram_tensor("iota4", [128, mxc, 128], BF16, kind="ExternalInput")
    out = nc.dram_tensor("out", [sh, cfg.h2], BF16, kind="ExternalOutput")

    rg = [list(range(cfg.cores))]
    stop = stop_after

    with tile.TileContext(nc) as tc:
        with (
            tc.tile_pool(name="constp", bufs=1) as constp,
            tc.tile_pool(name="dram", bufs=1, space="DRAM") as dram,
            tc.tile_pool(name="ohp", bufs=4) as ohp,
            tc.tile_pool(name="sp", bufs=4) as sp,
            tc.tile_pool(name="pp", bufs=6, space="PSUM") as pp,
        ):
            g1s0 = dram.tile([sh // 2, cfg.h1], BF16)
            g1s1 = dram.tile([sh // 2, cfg.h1], BF16)
            g1f0 = dram.tile([nrows_h, cfg.h1], BF16, addr_space=aspace)
            g1f1 = dram.tile([nrows_h, cfg.h1], BF16, addr_space=aspace)
            z1d0 = dram.tile([sh // 2, cfg.h1], BF16)
            z1d1 = dram.tile([sh // 2, cfg.h1], BF16)
            g2s0 = dram.tile([sh // 2, cfg.h2], BF16)
            g2s1 = dram.tile([sh // 2, cfg.h2], BF16)
            g2f0 = dram.tile([nrows_h, cfg.h2], BF16, addr_space=aspace)
            g2f1 = dram.tile([nrows_h, cfg.h2], BF16, addr_space=aspace)

            # ---- constants ----
            w1sb = constp.tile([128, kin, cfg.h1], BF16)
            for k in range(kin):
                nc.sync.dma_start(w1sb[:, k, :], w1[k * 128:(k + 1) * 128, :])
            w2sb = constp.tile([128, kh1, cfg.h2], BF16)
            for k in range(kh1):
                nc.sync.dma_start(w2sb[:, k, :], w2[k * 128:(k + 1) * 128, :])
            idxsb = constp.tile([128, meta.tot_cols], I16)
            nc.sync.dma_start(idxsb[:], idx[:])
            dstsb = constp.tile([128, cfg.bpc * nch], BF16)
            nc.sync.dma_start(dstsb[:], dstloc[:])
            dvsb = constp.tile([128, cfg.bpc], FP32)
            nc.sync.dma_start(dvsb[:], dinv[:])
            d08sb = constp.tile([128, cfg.bpc], FP32)
            nc.sync.dma_start(d08sb[:], dinv08[:])
            d02sb = constp.tile([128, cfg.bpc], FP32)
            nc.sync.dma_start(d02sb[:], dinv02[:])
            iotasb = constp.tile([128, mxc, 128], BF16)
            nc.sync.dma_start(iotasb[:], iota_d[:])

            # ---- dense phase helper: g = dinv * (inT-tiles @ W) ----
            def dense(insb, wsb, kk, h, sink, b0, b1):
                for b in range(b0, b1):
                    rb = b - b0
                    ps = pp.tile([128, h], FP32, tag="ps")
                    for k in range(kk):
                        nc.tensor.matmul(ps[:], insb[:, k, rb * 128:(rb + 1) * 128],
                                         wsb[:, k, :],
                                         start=(k == 0), stop=(k == kk - 1))
                    gt = sp.tile([128, h], BF16, tag="gt")
                    nc.scalar.mul(gt[:], ps[:], dvsb[:, b:b + 1])
                    nc.sync.dma_start(sink[rb * 128:(rb + 1) * 128, :], gt[:])

            qctr = [0, 0]

            def gather_grp(pool, src_ap, g, side, h):
                """one dma_gather call for a whole (group, side)."""
                (g0, gn, col_a, col_b) = meta.groups[g]
                cp = cpa if side == 0 else cpb
                if cp == 0:
                    return None
                col0 = col_a if side == 0 else col_b
                t = pool.tile([128, cfg.grp * cp, h], BF16,
                              tag=f"g{side}")
                nck = gn * cp
                for off in range(0, nck, cfg.mcall):
                    nn = min(cfg.mcall, nck - off)
                    nc.gpsimd.dma_gather(
                        t[:, off:off + nn, :], src_ap,
                        idxsb[:, col0 + off * 8: col0 + (off + nn) * 8],
                        nn * 128, nn * 128, h,
                        queue_num=side * 2 + qctr[side] % 2)
                    qctr[side] += 1
                return t

            def message_pass(gpa, gpb, gla, glb, h, z_sink, after_grp=None):
                ngr = len(meta.groups)
                look = min(cfg.look, ngr)
                ga_q = [gather_grp(gpa, gla, g, 0, h) for g in range(look)]
                for g, (g0, gn, col_a, col_b) in enumerate(meta.groups):
                    gb = gather_grp(gpb, glb, g, 1, h)
                    ga = ga_q[0] if ga_q else None
                    if ga_q:
                        ga_q.pop(0)
                    if g + look < ngr:
                        ga_q.append(gather_grp(gpa, gla, g + look, 0, h))
                    for j in range(gn):
                        b = g0 + j
                        ps = pp.tile([128, h], FP32, tag="ps")
                        for s, (cp, gt) in enumerate(((cpa, ga), (cpb, gb))):
                            if cp == 0:
                                continue
                            oh = ohp.tile([128, mxc, 128], BF16, tag="oh")
                            dcol = dstsb[:, b * nch + s * cpa:
                                         b * nch + s * cpa + cp]
                            nc.vector.tensor_tensor(
                                oh[:, 0:cp, :], iotasb[:, 0:cp, :],
                                dcol.broadcast_to([128, cp, 128]),
                                op=OP.is_equal)
                            for c in range(cp):
                                cc = c + s * cpa
                                nc.tensor.matmul(ps[:], oh[:, c, :],
                                                 gt[:, j * cp + c, :],
                                                 start=(cc == 0),
                                                 stop=(cc == nch - 1))
                        z_sink(b, ps)
                    if after_grp is not None and g in after_grp:
                        after_grp[g]()

            def z1_sink(b, acc):
                r = sp.tile([128, cfg.h1], FP32, tag="r")
                nc.scalar.activation(r[:], acc[:], AT.Relu,
                                     bias=0.0, scale=d08sb[:, b:b + 1])
                z = sp.tile([128, cfg.h1], BF16, tag="z1")
                nc.vector.scalar_tensor_tensor(z[:], acc[:], d02sb[:, b:b + 1],
                                               r[:], op0=OP.mult, op1=OP.add)
                zt, rb = (z1d0, b) if b < hb else (z1d1, b - hb)
                nc.sync.dma_start(zt[rb * 128:(rb + 1) * 128, :], z[:])

            def out_sink(b, acc):
                r = sp.tile([128, cfg.h2], FP32, tag="r")
                nc.scalar.activation(r[:], acc[:], AT.Relu,
                                     bias=0.0, scale=d08sb[:, b:b + 1])
                z = sp.tile([128, cfg.h2], BF16, tag="zo")
                nc.vector.scalar_tensor_tensor(z[:], acc[:], d02sb[:, b:b + 1],
                                               r[:], op0=OP.mult, op1=OP.add)
                nc.sync.dma_start(out[b * 128:(b + 1) * 128, :], z[:])

            # ---- phase 1: g1 shard (two halves, allgathered separately) ----
            with tc.tile_pool(name="xtp", bufs=1) as xtp:
                xt0 = xtp.tile([128, kin, hb * 128], BF16)
                xt1 = xtp.tile([128, kin, sh - hb * 128], BF16)
                for k in range(kin):
                    nc.sync.dma_start(xt0[:, k, :],
                                      xt[k * 128:(k + 1) * 128, 0:hb * 128])
                    nc.sync.dma_start(xt1[:, k, :],
                                      xt[k * 128:(k + 1) * 128, hb * 128:])
                dense(xt0, w1sb, kin, cfg.h1, g1s0, 0, hb)
                if stop != "p1":
                    nc.gpsimd.collective_compute(
                        "AllGather", OP.bypass, replica_groups=rg,
                        ins=[g1s0.opt()], outs=[g1f0.opt()])
                dense(xt1, w1sb, kin, cfg.h1, g1s1, hb, cfg.bpc)
                if stop != "p1":
                    nc.gpsimd.collective_compute(
                        "AllGather", OP.bypass, replica_groups=rg,
                        ins=[g1s1.opt()], outs=[g1f1.opt()])

            if stop in ("p1", "ag1"):
                nc.compile()
                return nc

            # dense2 + AG for a z1 half (injected into the MP1 loop)
            def dense2_half(half, ztp):
                zt = z1d0 if half == 0 else z1d1
                gs = g2s0 if half == 0 else g2s1
                z1t = ztp.tile([128, kh1, sh // 2], BF16, tag="z1t")
                for k in range(kh1):
                    nc.sync.dma_start_transpose(
                        out=z1t[:, k, :],
                        in_=zt[:, k * 128:(k + 1) * 128])
                dense(z1t, w2sb, kh1, cfg.h2, gs,
                      half * hb, half * hb + hb)

            def ag2_half(half):
                gs, gf = (g2s0, g2f0) if half == 0 else (g2s1, g2f1)
                nc.gpsimd.collective_compute(
                    "AllGather", OP.bypass, replica_groups=rg,
                    ins=[gs.opt()], outs=[gf.opt()])

            # ---- phase 3: layer-1 MP with dense2/AG injected ----
            mid = hb // cfg.grp            # group finishing block hb-1
            with tc.tile_pool(name="gp1a", bufs=cfg.look + 2) as gp1a, \
                    tc.tile_pool(name="gp1b", bufs=3) as gp1b, \
                    tc.tile_pool(name="ztp", bufs=2) as ztp:
                after = {
                    mid: lambda: dense2_half(0, ztp),
                    mid + 2: lambda: ag2_half(0),
                    len(meta.groups) - 1: lambda: (dense2_half(1, ztp),
                                                   ag2_half(1)),
                }
                if stop == "p3":
                    after = None
                message_pass(gp1a, gp1b, g1f0[:, :], g1f1[:, :], cfg.h1,
                             z1_sink, after_grp=after)

            if stop in ("p3", "p4"):
                nc.compile()
                return nc

            # ---- phase 6: layer-2 message passing -> out ----
            with tc.tile_pool(name="gp2a", bufs=cfg.look + 2) as gp2a, \
                    tc.tile_pool(name="gp2b", bufs=3) as gp2b:
                message_pass(gp2a, gp2b, g2f0[:, :], g2f1[:, :], cfg.h2,
                             out_sink)

    nc.compile()
    return nc


def install_ntff_hook():
    """The agent image's antenv lacks axon_hooks; graft it so trace=True
    can reach the libaxon_pjrt NTFF profiling C ABI."""
    import sys as _sys, types as _types
    if "antenv.axon_hooks" in _sys.modules:
        return
    _sys.path.insert(0, "/root/.axon_site")
    from trn_agent_boot.trn_boot import _ntff_profile_via_ctypes
    hook = _ntff_profile_via_ctypes("/opt/axon/libaxon_pjrt.so")
    mod = _types.ModuleType("antenv.axon_hooks")
    mod._hook = hook
    mod.get_axon_ntff_profile_hook = lambda: mod._hook
    mod.set_axon_ntff_profile_hook = lambda h: setattr(mod, "_hook", h)
    _sys.modules["antenv.axon_hooks"] = mod
    import antenv
    antenv.axon_hooks = mod


def run(cfg: Cfg, X, edge_index, W1, b1, W2, b2, trace=False,
        stop_after='full', trace_cores=None):
    if trace:
        install_ntff_hook()
    import time
    t0 = time.time()
    in_maps, meta, slot = preprocess(cfg, X, edge_index, W1, b1, W2, b2)
    t1 = time.time()
    nc = build(cfg, meta, stop_after=stop_after)
    t2 = time.time()
    print(f"preprocess {t1-t0:.1f}s, build+compile {t2-t1:.1f}s", flush=True)
    res = run_bass_kernel_spmd(nc, in_maps, core_ids=list(range(cfg.cores)),
                               trace=trace, trace_cores=trace_cores)
    print(f"hw run {time.time()-t2:.1f}s", flush=True)
    nblk = cfg.npad // 128
    full = np.empty((cfg.npad, cfg.h2), np.float32)
    for c in range(cfg.cores):
        o = np.asarray(res.results[c]["out"], dtype=np.float32)
        for p, b in enumerate(range(c, nblk, cfg.cores)):
            full[b * 128:(b + 1) * 128] = o[p * 128:(p + 1) * 128]
    full = full[slot]
    return full, res, nc, in_maps, meta


import os
_CFG = Cfg(n=50000, e=800000, d_in=512, h1=256, h2=128, cores=8, bpc=50,
           scratch=int(os.environ.get("K_SCRATCH", "16384")),
           mcall=int(os.environ.get("K_MCALL", "8")),
           look=int(os.environ.get("K_LOOK", "3")),
           shared_ag=bool(int(os.environ.get("K_SHARED_AG", "0"))))


def kernel(X, edge_index, W1, b1, W2, b2):
    full, _res, _nc, _maps, _meta = run(
        _CFG, X, edge_index, W1, b1, W2, b2, trace=False)
    return full


# revision 8
# speedup vs baseline: 1.2174x; 1.2174x over previous
"""Self-contained Trainium2 Bass kernel for a 2-layer GCN encoder
(PyG GCNConv x2 with LeakyReLU), distributed over 8 NeuronCores.

kernel(**inputs) takes the full unsharded inputs (X [50000,512] f32,
edge_index [2,800000] int64, W1/b1/W2/b2) and returns the full
[50000,128] f32 output. See build() for the device program.

v2 changes vs baseline:
- host-side load balancing: nodes are assigned to 128-row dst blocks so
  per-(block, src-half) edge counts are even -> fewer gather chunks
- one dma_gather call per (group, side) (amortizes the ~1us fixed SWDGE
  cost; needs a larger dynamic-dma descriptor ring)
- z sinks use a single fused Lrelu activation on the Scalar engine
- dense2+allgather for each z1 half are issued inside the layer-1
  message-passing loop so the collectives overlap compute
- A-side gathers run a few groups ahead of B-side gathers so a pending
  allgather of the B table never stalls the gpsimd gather stream
"""

import sys
if "/opt/trn_rl_repo" not in sys.path:
    sys.path.insert(0, "/opt/trn_rl_repo")

import math
from dataclasses import dataclass, field

import numpy as np
import ml_dtypes

import concourse.bass as bass
import concourse.tile as tile
from concourse import bacc, mybir
from concourse.bass_utils import run_bass_kernel_spmd

FP32 = mybir.dt.float32
BF16 = mybir.dt.bfloat16
I32 = mybir.dt.int32
I16 = mybir.dt.int16


@dataclass
class Cfg:
    n: int          # real node count
    e: int          # real edge count
    d_in: int
    h1: int
    h2: int
    cores: int = 8
    bpc: int = 50   # 128-node dst blocks per core
    grp: int = 2    # dst blocks per dma_gather call
    neg: float = 0.2
    look: int = 3   # A-gather lookahead (groups)
    shared_ag: bool = False
    scratch: int = 16384  # dynamic dma descriptor ring: scratch//16 descs
    mcall: int = 8        # max 128-idx chunks per dma_gather call

    @property
    def npad(self):
        return self.cores * self.bpc * 128

    @property
    def shard(self):
        return self.bpc * 128


@dataclass
class Meta:
    cpa: int  # chunks per block, side A (src half 0)
    cpb: int  # chunks per block, side B
    tot_cols: int = 0   # idx tile columns (int16 packed by 16)
    nch: int = 0        # chunks per block total


def _balance_slots(cfg: Cfg, src, dst):
    """Assign nodes to slots so per-(block, src-half) in-edge counts are
    balanced. Returns slot_of_node [n] -> slot in [0, npad)."""
    n, npad = cfg.n, cfg.npad
    nblk = npad // 128
    hb = cfg.bpc // 2
    half_blocks = nblk // 2  # blocks 0..half_blocks-1 are half A

    indeg = np.bincount(dst, minlength=n).astype(np.int64)

    # step 1: split nodes into two halves with ~equal total indegree
    order = np.argsort(-indeg, kind="stable")
    half_of_node = np.zeros(n, np.int8)
    # snake: 0,1,1,0,0,1,1,0...
    snake = np.tile([0, 1, 1, 0], n // 4 + 1)[:n]
    half_of_node[order] = snake
    # fix cardinality: each half can hold npad//2 slots; count real nodes
    cnt0 = int((half_of_node == 0).sum())
    cap = npad // 2
    if cnt0 > cap:
        movers = np.where(half_of_node == 0)[0][: cnt0 - cap]
        half_of_node[movers] = 1
    elif n - cnt0 > cap:
        movers = np.where(half_of_node == 1)[0][: (n - cnt0) - cap]
        half_of_node[movers] = 0

    # per-dst-node in-edge counts split by src half (+1 self edge, own half)
    sh = half_of_node[src]
    a_n = np.bincount(dst[sh == 0], minlength=n).astype(np.int64)
    b_n = np.bincount(dst[sh == 1], minlength=n).astype(np.int64)

    slot_of_node = np.empty(n, np.int64)
    # physical block ids: half A blocks 0..199 (core nb%8, p=nb//8 < hb)
    for H in (0, 1):
        nodes = np.where(half_of_node == H)[0]
        aa = a_n[nodes] + (1 if H == 0 else 0)   # self edge on own side
        bb = b_n[nodes] + (1 if H == 1 else 0)
        w = aa + bb
        srt = np.argsort(-w, kind="stable")
        nodes, aa, bb = nodes[srt], aa[srt], bb[srt]
        nb0 = 0 if H == 0 else half_blocks
        cA = np.zeros(half_blocks, np.int64)
        cB = np.zeros(half_blocks, np.int64)
        fill = np.zeros(half_blocks, np.int64)
        lane = np.empty(len(nodes), np.int64)
        blk = np.empty(len(nodes), np.int64)
        for i in range(len(nodes)):
            score = np.maximum(cA + aa[i], cB + bb[i]).astype(np.float64)
            score[fill >= 128] = np.inf
            j = int(np.argmin(score))
            blk[i] = j
            lane[i] = fill[j]
            fill[j] += 1
            cA[j] += aa[i]
            cB[j] += bb[i]
        slot_of_node[nodes] = (nb0 + blk) * 128 + lane
    return slot_of_node


def preprocess(cfg: Cfg, X, edge_index, W1, b1, W2, b2):
    """Host-side: shard + edge partitioning. Returns (in_maps, meta)."""
    n, npad = cfg.n, cfg.npad
    src = np.asarray(edge_index[0], dtype=np.int64)
    dst = np.asarray(edge_index[1], dtype=np.int64)
    assert not np.any(np.asarray(b1)) and not np.any(np.asarray(b2)), \
        "nonzero bias unsupported in this build"

    slot = _balance_slots(cfg, src, dst)

    deg_n = np.bincount(dst, minlength=n).astype(np.float32) + 1.0
    dinv = np.ones(npad, np.float32)
    dinv[slot] = (1.0 / np.sqrt(deg_n)).astype(np.float32)

    # self loops for every real node; edges in slot space
    asrc = slot[np.concatenate([src, np.arange(n, dtype=np.int64)])]
    adst = slot[np.concatenate([dst, np.arange(n, dtype=np.int64)])]

    # gather-table row of a slot: tables are the AG concat of core shards
    hb = cfg.bpc // 2
    nb_ = np.arange(npad, dtype=np.int64) >> 7
    c_ = nb_ % cfg.cores
    p_ = nb_ // cfg.cores
    half_ = (p_ >= hb).astype(np.int64)
    perm = (half_ * (npad // 2) + c_ * (hb * 128) + (p_ - half_ * hb) * 128
            + (np.arange(npad, dtype=np.int64) & 127))
    asrc = perm[asrc]

    blk = adst >> 7
    side = (asrc >= npad // 2).astype(np.int64)
    order = np.lexsort((asrc, side, blk))
    asrc, adst, blk, side = asrc[order], adst[order], blk[order], side[order]

    nblk = npad // 128
    cnt_a = np.bincount(blk[side == 0], minlength=nblk)
    cnt_b = np.bincount(blk[side == 1], minlength=nblk)
    cpa = int(math.ceil(cnt_a.max() / 128)) if cnt_a.max() > 0 else 0
    cpb = int(math.ceil(cnt_b.max() / 128)) if cnt_b.max() > 0 else 0
    cap_a, cap_b = cpa * 128, cpb * 128
    nch = cpa + cpb

    nrows_h = npad // 2
    spread = (np.arange(max(cap_a, cap_b, 1), dtype=np.int64) * 67)
    idx_a = np.ascontiguousarray(
        ((np.arange(nblk)[:, None] * 997 + spread[None, :cap_a]) % nrows_h
         ).astype(np.int16)) if cap_a else np.zeros((nblk, 1), np.int16)
    idx_b = np.ascontiguousarray(
        ((np.arange(nblk)[:, None] * 997 + spread[None, :cap_b]) % nrows_h
         ).astype(np.int16)) if cap_b else np.zeros((nblk, 1), np.int16)
    assert cfg.bpc % 2 == 0
    dstloc = np.full((nblk, nch * 128), -1, np.int32)

    for s, (idx_t, cnt, coff, roff) in enumerate(
            ((idx_a, cnt_a, 0, 0), (idx_b, cnt_b, cap_a, npad // 2))):
        mask = side == s
        b_, s_, d_ = blk[mask], asrc[mask], adst[mask]
        start = np.zeros(nblk + 1, np.int64)
        np.cumsum(cnt, out=start[1:])
        pos = np.arange(len(b_)) - start[b_]
        idx_t[b_, pos] = (s_ - roff).astype(np.int16)
        dstloc[b_, coff + pos] = (d_ & 127).astype(np.int32)

    tot_cols = cfg.bpc * (cap_a + cap_b) // 16
    meta = Meta(cpa=cpa, cpb=cpb, tot_cols=tot_cols, nch=nch)

    # replicated / per-core tensors
    XT = np.zeros((cfg.d_in, npad), np.float32)
    XT[:, slot] = np.asarray(X, np.float32).T
    XT = XT.astype(ml_dtypes.bfloat16)
    W1b = np.asarray(W1, np.float32).astype(ml_dtypes.bfloat16)
    W2b = np.asarray(W2, np.float32).astype(ml_dtypes.bfloat16)
    mxc = max(cpa, cpb, 1)
    iota4 = np.ascontiguousarray(np.broadcast_to(
        np.arange(128, dtype=np.float32)[None, None, :],
        (128, mxc, 128))).astype(ml_dtypes.bfloat16)

    in_maps = []
    for c in range(cfg.cores):
        blocks = list(range(c, nblk, cfg.cores))   # round-robin assignment
        # side-major streams: [all blocks' A chunks | all blocks' B chunks]
        flat = np.concatenate([idx_a[blocks, :cap_a].reshape(-1),
                               idx_b[blocks, :cap_b].reshape(-1)])
        assert flat.size == tot_cols * 16, (flat.size, tot_cols * 16)
        idx_tile = np.ascontiguousarray(
            np.tile(flat.reshape(-1, 16).T, (8, 1)))          # [128, tot_cols]

        dst_tile = np.ascontiguousarray(
            dstloc[blocks].reshape(cfg.bpc * nch, 128).T)      # [128, bpc*nch]

        node_sel = (np.asarray(blocks)[:, None] * 128
                    + np.arange(128)[None, :]).reshape(-1)
        dv = dinv[node_sel].reshape(cfg.bpc, 128).T
        m = {
            "xt": np.ascontiguousarray(XT[:, node_sel]),
            "w1": W1b, "w2": W2b,
            "idx": idx_tile,
            "dstloc": dst_tile.astype(np.float32).astype(ml_dtypes.bfloat16),
            "dinv": np.ascontiguousarray(dv).astype(np.float32),
            "dinv08": np.ascontiguousarray(dv * (1.0 - cfg.neg)).astype(np.float32),
            "dinv02": np.ascontiguousarray(dv * cfg.neg).astype(np.float32),
            "iota4": iota4,
        }
        in_maps.append(m)
    return in_maps, meta, slot


def build(cfg: Cfg, meta: Meta, stop_after: str = 'full'):
    nc = bacc.Bacc("TRN2", target_bir_lowering=False, debug=False,
                   num_devices=cfg.cores, num_swdge_queues=4,
                   dynamic_dma_scratch_size=cfg.scratch)
    sh, npad = cfg.shard, cfg.npad
    kin, kh1 = cfg.d_in // 128, cfg.h1 // 128
    cpa, cpb, nch = meta.cpa, meta.cpb, meta.nch
    mxc = max(cpa, cpb, 1)
    nrows_h = npad // 2
    hb = cfg.bpc // 2
    AT = mybir.ActivationFunctionType
    OP = mybir.AluOpType
    aspace = "Shared" if cfg.shared_ag else "Local"

    xt = nc.dram_tensor("xt", [cfg.d_in, sh], BF16, kind="ExternalInput")
    w1 = nc.dram_tensor("w1", [cfg.d_in, cfg.h1], BF16, kind="ExternalInput")
    w2 = nc.dram_tensor("w2", [cfg.h1, cfg.h2], BF16, kind="ExternalInput")
    idx = nc.dram_tensor("idx", [128, meta.tot_cols], I16, kind="ExternalInput")
    dstloc = nc.dram_tensor("dstloc", [128, cfg.bpc * nch], BF16, kind="ExternalInput")
    dinv = nc.dram_tensor("dinv", [128, cfg.bpc], FP32, kind="ExternalInput")
    dinv08 = nc.dram_tensor("dinv08", [128, cfg.bpc], FP32, kind="ExternalInput")
    dinv02 = nc.dram_tensor("dinv02", [128, cfg.bpc], FP32, kind="ExternalInput")
    iota_d = nc.dram_tensor("iota4", [128, mxc, 128], BF16, kind="ExternalInput")
    out = nc.dram_tensor("out", [sh, cfg.h2], BF16, kind="ExternalOutput")

    rg = [list(range(cfg.cores))]
    stop = stop_after

    with tile.TileContext(nc) as tc:
        with (
            tc.tile_pool(name="constp", bufs=1) as constp,
            tc.tile_pool(name="dram", bufs=1, space="DRAM") as dram,
            tc.tile_pool(name="ohp", bufs=4) as ohp,
            tc.tile_pool(name="sp", bufs=4) as sp,
            tc.tile_pool(name="pp", bufs=6, space="PSUM") as pp,
        ):
            g1s0 = dram.tile([sh // 2, cfg.h1], BF16)
            g1s1 = dram.tile([sh // 2, cfg.h1], BF16)
            g1f0 = dram.tile([nrows_h, cfg.h1], BF16, addr_space=aspace)
            g1f1 = dram.tile([nrows_h, cfg.h1], BF16, addr_space=aspace)
            z1d0 = dram.tile([sh // 2, cfg.h1], BF16)
            z1d1 = dram.tile([sh // 2, cfg.h1], BF16)
            g2s0 = dram.tile([sh // 2, cfg.h2], BF16)
            g2s1 = dram.tile([sh // 2, cfg.h2], BF16)
            g2f0 = dram.tile([nrows_h, cfg.h2], BF16, addr_space=aspace)
            g2f1 = dram.tile([nrows_h, cfg.h2], BF16, addr_space=aspace)

            # ---- constants ----
            w1sb = constp.tile([128, kin, cfg.h1], BF16)
            for k in range(kin):
                nc.sync.dma_start(w1sb[:, k, :], w1[k * 128:(k + 1) * 128, :])
            w2sb = constp.tile([128, kh1, cfg.h2], BF16)
            for k in range(kh1):
                nc.sync.dma_start(w2sb[:, k, :], w2[k * 128:(k + 1) * 128, :])
            idxsb = constp.tile([128, meta.tot_cols], I16)
            nc.sync.dma_start(idxsb[:], idx[:])
            dstsb = constp.tile([128, cfg.bpc * nch], BF16)
            nc.sync.dma_start(dstsb[:], dstloc[:])
            dvsb = constp.tile([128, cfg.bpc], FP32)
            nc.sync.dma_start(dvsb[:], dinv[:])
            d08sb = constp.tile([128, cfg.bpc], FP32)
            nc.sync.dma_start(d08sb[:], dinv08[:])
            d02sb = constp.tile([128, cfg.bpc], FP32)
            nc.sync.dma_start(d02sb[:], dinv02[:])
            iotasb = constp.tile([128, mxc, 128], BF16)
            nc.sync.dma_start(iotasb[:], iota_d[:])

            # ---- dense phase helper: g = dinv * (inT-tiles @ W) ----
            def dense(insb, wsb, kk, h, sink, b0, b1):
                for b in range(b0, b1):
                    rb = b - b0
                    ps = pp.tile([128, h], FP32, tag="ps")
                    for k in range(kk):
                        nc.tensor.matmul(ps[:], insb[:, k, rb * 128:(rb + 1) * 128],
                                         wsb[:, k, :],
                                         start=(k == 0), stop=(k == kk - 1))
                    gt = sp.tile([128, h], BF16, tag="gt")
                    nc.scalar.mul(gt[:], ps[:], dvsb[:, b:b + 1])
                    nc.sync.dma_start(sink[rb * 128:(rb + 1) * 128, :], gt[:])

            qctr = [0, 0]

            def gather_grp(pool, src_ap, g, side, h):
                """one dma_gather call for a whole (group, side)."""
                (g0, gn, col_a, col_b) = meta.groups[g]
                cp = cpa if side == 0 else cpb
                if cp == 0:
                    return None
                col0 = col_a if side == 0 else col_b
                t = pool.tile([128, cfg.grp * cp, h], BF16,
                              tag=f"g{side}")
                nck = gn * cp
                for off in range(0, nck, cfg.mcall):
                    nn = min(cfg.mcall, nck - off)
                    nc.gpsimd.dma_gather(
                        t[:, off:off + nn, :], src_ap,
                        idxsb[:, col0 + off * 8: col0 + (off + nn) * 8],
                        nn * 128, nn * 128, h,
                        queue_num=side * 2 + qctr[side] % 2)
                    qctr[side] += 1
                return t

            def message_pass(gpa, gpb, gla, glb, h, z_sink, after_grp=None):
                ngr = len(meta.groups)
                look = min(cfg.look, ngr)
                ga_q = [gather_grp(gpa, gla, g, 0, h) for g in range(look)]
                for g, (g0, gn, col_a, col_b) in enumerate(meta.groups):
                    gb = gather_grp(gpb, glb, g, 1, h)
                    ga = ga_q[0] if ga_q else None
                    if ga_q:
                        ga_q.pop(0)
                    if g + look < ngr:
                        ga_q.append(gather_grp(gpa, gla, g + look, 0, h))
                    for j in range(gn):
                        b = g0 + j
                        ps = pp.tile([128, h], FP32, tag="ps")
                        for s, (cp, gt) in enumerate(((cpa, ga), (cpb, gb))):
                            if cp == 0:
                                continue
                            oh = ohp.tile([128, mxc, 128], BF16, tag="oh")
                            dcol = dstsb[:, b * nch + s * cpa:
                                         b * nch + s * cpa + cp]
                            nc.vector.tensor_tensor(
                                oh[:, 0:cp, :], iotasb[:, 0:cp, :],
                                dcol.broadcast_to([128, cp, 128]),
                                op=OP.is_equal)
                            for c in range(cp):
                                cc = c + s * cpa
                                nc.tensor.matmul(ps[:], oh[:, c, :],
                                                 gt[:, j * cp + c, :],
                                                 start=(cc == 0),
                                                 stop=(cc == nch - 1))
                        z_sink(b, ps)
                    if after_grp is not None and g in after_grp:
                        after_grp[g]()

            def z1_sink(b, acc):
                r = sp.tile([128, cfg.h1], FP32, tag="r")
                nc.scalar.activation(r[:], acc[:], AT.Relu,
                                     bias=0.0, scale=d08sb[:, b:b + 1])
                z = sp.tile([128, cfg.h1], BF16, tag="z1")
                nc.vector.scalar_tensor_tensor(z[:], acc[:], d02sb[:, b:b + 1],
                                               r[:], op0=OP.mult, op1=OP.add)
                zt, rb = (z1d0, b) if b < hb else (z1d1, b - hb)
                nc.sync.dma_start(zt[rb * 128:(rb + 1) * 128, :], z[:])

            def out_sink(b, acc):
                r = sp.tile([128, cfg.h2], FP32, tag="r")
                nc.scalar.activation(r[:], acc[:], AT.Relu,
                                     bias=0.0, scale=d08sb[:, b:b + 1])
                z = sp.tile([128, cfg.h2], BF16, tag="zo")
                nc.vector.scalar_tensor_tensor(z[:], acc[:], d02sb[:, b:b + 1],
                                               r[:], op0=OP.mult, op1=OP.add)
                nc.sync.dma_start(out[b * 128:(b + 1) * 128, :], z[:])

            # ---- phase 1: g1 shard (two halves, allgathered separately) ----
            with tc.tile_pool(name="xtp", bufs=1) as xtp:
                xt0 = xtp.tile([128, kin, hb * 128], BF16)
                xt1 = xtp.tile([128, kin, sh - hb * 128], BF16)
                for k in range(kin):
                    nc.sync.dma_start(xt0[:, k, :],
                                      xt[k * 128:(k + 1) * 128, 0:hb * 128])
                    nc.sync.dma_start(xt1[:, k, :],
                                      xt[k * 128:(k + 1) * 128, hb * 128:])
                dense(xt0, w1sb, kin, cfg.h1, g1s0, 0, hb)
                if stop != "p1":
                    nc.gpsimd.collective_compute(
                        "AllGather", OP.bypass, replica_groups=rg,
                        ins=[g1s0.opt()], outs=[g1f0.opt()])
                dense(xt1, w1sb, kin, cfg.h1, g1s1, hb, cfg.bpc)
                if stop != "p1":
                    nc.gpsimd.collective_compute(
                        "AllGather", OP.bypass, replica_groups=rg,
                        ins=[g1s1.opt()], outs=[g1f1.opt()])

            if stop in ("p1", "ag1"):
                nc.compile()
                return nc

            # dense2 + AG for a z1 half (injected into the MP1 loop)
            def dense2_half(half, ztp):
                zt = z1d0 if half == 0 else z1d1
                gs = g2s0 if half == 0 else g2s1
                z1t = ztp.tile([128, kh1, sh // 2], BF16, tag="z1t")
                for k in range(kh1):
                    nc.sync.dma_start_transpose(
                        out=z1t[:, k, :],
                        in_=zt[:, k * 128:(k + 1) * 128])
                dense(z1t, w2sb, kh1, cfg.h2, gs,
                      half * hb, half * hb + hb)

            def ag2_half(half):
                gs, gf = (g2s0, g2f0) if half == 0 else (g2s1, g2f1)
                nc.gpsimd.collective_compute(
                    "AllGather", OP.bypass, replica_groups=rg,
                    ins=[gs.opt()], outs=[gf.opt()])

            # ---- phase 3: layer-1 MP with dense2/AG injected ----
            mid = hb // cfg.grp            # group finishing block hb-1
            with tc.tile_pool(name="gp1a", bufs=cfg.look + 2) as gp1a, \
                    tc.tile_pool(name="gp1b", bufs=3) as gp1b, \
                    tc.tile_pool(name="ztp", bufs=2) as ztp:
                after = {
                    mid: lambda: dense2_half(0, ztp),
                    mid + 2: lambda: ag2_half(0),
                    len(meta.groups) - 1: lambda: (dense2_half(1, ztp),
                                                   ag2_half(1)),
                }
                if stop == "p3":
                    after = None
                message_pass(gp1a, gp1b, g1f0[:, :], g1f1[:, :], cfg.h1,
                             z1_sink, after_grp=after)

            if stop in ("p3", "p4"):
                nc.compile()
                return nc

            # ---- phase 6: layer-2 message passing -> out ----
            with tc.tile_pool(name="gp2a", bufs=cfg.look + 2) as gp2a, \
                    tc.tile_pool(name="gp2b", bufs=3) as gp2b:
                message_pass(gp2a, gp2b, g2f0[:, :], g2f1[:, :], cfg.h2,
                             out_sink)

    nc.compile()
    return nc


def install_ntff_hook():
    """The agent image's antenv lacks axon_hooks; graft it so trace=True
    can reach the libaxon_pjrt NTFF profiling C ABI."""
    import sys as _sys, types as _types
    if "antenv.axon_hooks" in _sys.modules:
        return
    _sys.path.insert(0, "/root/.axon_site")
    from trn_agent_boot.trn_boot import _ntff_profile_via_ctypes
    hook = _ntff_profile_via_ctypes("/opt/axon/libaxon_pjrt.so")
    mod = _types.ModuleType("antenv.axon_hooks")
    mod._hook = hook
    mod.get_axon_ntff_profile_hook = lambda: mod._hook
    mod.set_axon_ntff_profile_hook = lambda h: setattr(mod, "_hook", h)
    _sys.modules["antenv.axon_hooks"] = mod
    import antenv
    antenv.axon_hooks = mod


def run(cfg: Cfg, X, edge_index, W1, b1, W2, b2, trace=False,
        stop_after='full', trace_cores=None):
    if trace:
        install_ntff_hook()
    import time
    t0 = time.time()
    in_maps, meta, slot = preprocess(cfg, X, edge_index, W1, b1, W2, b2)
    t1 = time.time()
    nc = build(cfg, meta, stop_after=stop_after)
    t2 = time.time()
    print(f"preprocess {t1-t0:.1f}s, build+compile {t2-t1:.1f}s", flush=True)
    res = run_bass_kernel_spmd(nc, in_maps, core_ids=list(range(cfg.cores)),
                               trace=trace, trace_cores=trace_cores)
    print(f"hw run {time.time()-t2:.1f}s", flush=True)
    nblk = cfg.npad // 128
    full = np.empty((cfg.npad, cfg.h2), np.float32)
    for c in range(cfg.cores):
        o = np.asarray(res.results[c]["out"], dtype=np.float32)
        for p, b in enumerate(range(c, nblk, cfg.cores)):
            full[b * 128:(b + 1) * 128] = o[p * 128:(p + 1) * 128]
    full = full[slot]
    return full, res, nc, in_maps, meta


import os
_CFG = Cfg(n=50000, e=800000, d_in=512, h1=256, h2=128, cores=8, bpc=50,
           scratch=int(os.environ.get("K_SCRATCH", "16384")),
           mcall=int(os.environ.get("K_MCALL", "8")),
           look=int(os.environ.get("K_LOOK", "3")),
           shared_ag=bool(int(os.environ.get("K_SHARED_AG", "0"))))


def kernel(X, edge_index, W1, b1, W2, b2):
    full, _res, _nc, _maps, _meta = run(
        _CFG, X, edge_index, W1, b1, W2, b2, trace=False)
    return full
